# revision 8
# baseline (speedup 1.0000x reference)
# Trainium2 Bass kernel for nn_FuzzyNeuralNework (moe_routing).
#
# Math (reference):
#   logits[b,r] = sum_d -(x[b,d]-cen[d,r])^2 / (2 sig[d,r]^2)
#   raw = exp(logits) * mask ;  frs = raw / (sum_r raw + 1e-10)
#   xn = batchnorm(x) (global batch stats, biased var)
#   out[b,c] = sum_r frs[b,r] * (xn @ W[r])[b,c] + sum_r frs[b,r]*bias[r,c]
#
# Kernel restructuring:
#   logits^T = A^T x2^T + Bc^T x^T + k,  A=-1/(2 sig^2), Bc=cen/sig^2,
#       k[r] = sum_d -cen^2/(2 sig^2)   (two PE matmuls in [r,b] layout)
#   normalization folded into the exp:  frs^T = exp(logits^T + k - ln(denom))
#       where denom = sum_r raw (K=1 PE matmul adds -ln(denom) to psum)
#   gating folded into the GEMM:  out^T[c,b] = sum_r W[r]^T @ (xn^T * frs^T[r,:])
#       accumulated over rules in PSUM (even rules / odd rules in 2 psum tiles)
#
# Sharding: batch B=8192 split across 8 cores (1024 each); small tensors
# replicated; BN stats computed on every core from the full (replicated) x^T.

import numpy as np

B, D, R, C = 8192, 128, 64, 64
NCORES = 8
BL = B // NCORES
BN_EPS = 1e-5

_CACHE = {}


def _build_bass():
    import concourse.bass as bass
    import concourse.tile as tile
    from concourse import bacc, mybir

    f32 = mybir.dt.float32
    bf16 = mybir.dt.bfloat16
    AF = mybir.ActivationFunctionType
    OP = mybir.AluOpType

    nc = bacc.Bacc(
        "TRN2", target_bir_lowering=False, debug=False, num_devices=NCORES
    )

    d_xtf = nc.dram_tensor("xt_full", [D, B], f32, kind="ExternalInput").ap()
    d_xtl = nc.dram_tensor("xt_loc", [D, BL], f32, kind="ExternalInput").ap()
    d_cen = nc.dram_tensor("centers_t", [D, R], f32, kind="ExternalInput").ap()
    d_sig = nc.dram_tensor("sigmas_t", [D, R], f32, kind="ExternalInput").ap()
    d_wst = nc.dram_tensor("wstack", [D, R * C], f32, kind="ExternalInput").ap()
    d_b2d = nc.dram_tensor("biases2d", [R, C], f32, kind="ExternalInput").ap()
    d_gam = nc.dram_tensor("gamma_c", [D, 1], f32, kind="ExternalInput").ap()
    d_bet = nc.dram_tensor("beta_c", [D, 1], f32, kind="ExternalInput").ap()
    d_msk = nc.dram_tensor("masks_c", [R, 1], f32, kind="ExternalInput").ap()
    d_out = nc.dram_tensor("outT", [C, BL], f32, kind="ExternalOutput").ap()

    with tile.TileContext(nc) as tc:
        with (
            tc.tile_pool(name="singles", bufs=1) as singles,
            tc.tile_pool(name="bigs", bufs=1) as bigs,
            tc.tile_pool(name="gpool", bufs=6) as gpool,
        ):
            ps_early_cm = tc.tile_pool(name="ps_early", bufs=1, space="PSUM")
            ps_small = ps_early_cm.__enter__()
            ps_logp = ps_small
            # ---- PE warmup (HAM) while DMAs stream in -------------------
            warm = singles.tile([D, 128], bf16)
            nc.vector.memset(warm, 0.0)
            warm_ps = ps_small.tile([D, 128], f32)
            for _ in range(24):
                nc.tensor.matmul(warm_ps, warm, warm, start=True, stop=True)

            # ---- load inputs -------------------------------------------
            sb_cen = singles.tile([D, R], f32)
            sb_sig = singles.tile([D, R], f32)
            sb_gam = singles.tile([D, 1], f32)
            sb_bet = singles.tile([D, 1], f32)
            sb_msk = singles.tile([R, 1], f32)
            sb_b2d = singles.tile([R, C], f32)
            nc.sync.dma_start(out=sb_cen, in_=d_cen)
            nc.sync.dma_start(out=sb_sig, in_=d_sig)
            nc.sync.dma_start(out=sb_gam, in_=d_gam)
            nc.sync.dma_start(out=sb_bet, in_=d_bet)
            nc.sync.dma_start(out=sb_msk, in_=d_msk)
            nc.sync.dma_start(out=sb_b2d, in_=d_b2d)

            sb_xtl = bigs.tile([D, BL], f32)
            nc.sync.dma_start(out=sb_xtl, in_=d_xtl)
            sb_xtf = bigs.tile([D, B], f32)
            for h in range(4):
                sl = slice(h * (B // 4), (h + 1) * (B // 4))
                nc.sync.dma_start(out=sb_xtf[:, sl], in_=d_xtf[:, sl])
            sb_wst = bigs.tile([D, R * C], f32)
            for h in range(2):
                sl = slice(h * (R * C // 2), (h + 1) * (R * C // 2))
                nc.sync.dma_start(out=sb_wst[:, sl], in_=d_wst[:, sl])

            # ---- Gaussian-membership coefficient prep (tiny DVE ops) ----
            sigsq = singles.tile([D, R], f32)
            nc.vector.tensor_mul(sigsq, sb_sig, sb_sig)
            recs = singles.tile([D, R], f32)
            nc.vector.reciprocal(recs, sigsq)
            sbA = singles.tile([D, R], f32)
            nc.vector.tensor_scalar_mul(sbA, recs, -0.5)
            sbBc = singles.tile([D, R], f32)
            nc.vector.tensor_mul(sbBc, sb_cen, recs)
            csq = singles.tile([D, R], f32)
            nc.vector.tensor_mul(csq, sb_cen, sb_cen)
            cA = singles.tile([D, R], f32)
            nc.vector.tensor_mul(cA, csq, sbA)

            ones_d = singles.tile([D, 1], f32)
            nc.vector.memset(ones_d, 1.0)
            ps_k = ps_small.tile([R, 1], f32)
            nc.tensor.matmul(ps_k, cA, ones_d, start=True, stop=True)
            sb_k = singles.tile([R, 1], f32)
            nc.vector.tensor_copy(sb_k, ps_k)

            # ---- BN stats over the full batch (replicated) --------------
            # sum(x^2) via one ACT Square pass with accumulate; the squared
            # output itself is scratch (bf16 to halve the write bandwidth).
            sq_scratch = bigs.tile([D, B], bf16)
            sq_sum = singles.tile([D, 1], f32)
            nc.scalar.activation(
                out=sq_scratch, in_=sb_xtf, func=AF.Square, accum_out=sq_sum
            )
            # sum(x) via a second ACT pass (Identity + accumulate), reusing
            # the same scratch output tile.
            x_sum = singles.tile([D, 1], f32)
            nc.scalar.activation(
                out=sq_scratch, in_=sb_xtf, func=AF.Identity, accum_out=x_sum
            )
            mean = singles.tile([D, 1], f32)
            nc.vector.tensor_scalar_mul(mean, x_sum, 1.0 / float(B))
            var = singles.tile([D, 1], f32)
            msq = singles.tile([D, 1], f32)
            nc.vector.tensor_mul(msq, mean, mean)
            nc.vector.tensor_scalar_mul(var, sq_sum, 1.0 / float(B))
            nc.vector.tensor_sub(var, var, msq)
            # rstd = exp(-0.5 * ln(var + eps)) : avoids the low-precision
            # Rsqrt table and shares the natural_log_exp ACT table set.
            eps_d = singles.tile([D, 1], f32)
            nc.vector.memset(eps_d, float(BN_EPS))
            lnv = singles.tile([D, 1], f32)
            nc.scalar.activation(lnv, var, AF.Ln, bias=eps_d)
            rstd = singles.tile([D, 1], f32)
            nc.scalar.activation(rstd, lnv, AF.Exp, scale=-0.5)
            a_sc = singles.tile([D, 1], f32)
            nc.vector.tensor_mul(a_sc, rstd, sb_gam)
            mu_a = singles.tile([D, 1], f32)
            nc.vector.tensor_mul(mu_a, mean, a_sc)
            c0 = singles.tile([D, 1], f32)
            nc.vector.tensor_sub(c0, sb_bet, mu_a)

            xn_bf = bigs.tile([D, BL], bf16)
            nc.vector.tensor_scalar(
                out=xn_bf, in0=sb_xtl, scalar1=a_sc, scalar2=c0,
                op0=OP.mult, op1=OP.add,
            )

            # ---- logits^T in PSUM [R, BL] (fp32 matmuls: exp-sensitive) --
            xsq_l = bigs.tile([D, BL], f32)
            nc.scalar.activation(xsq_l, sb_xtl, AF.Square)
            ps_log = ps_logp.tile([R, BL], f32)
            for h in range(2):
                sl = slice(h * 512, (h + 1) * 512)
                nc.tensor.matmul(
                    ps_log[:, sl], sbA, xsq_l[:, sl], start=True, stop=False
                )
                nc.tensor.matmul(
                    ps_log[:, sl], sbBc, sb_xtl[:, sl], start=False, stop=True
                )

            # raw = exp(logits + k) * mask   (fp32, matches reference
            # underflow behaviour; no max-subtraction on purpose)
            raw = bigs.tile([R, BL], f32)
            nc.scalar.activation(raw, ps_log, AF.Exp, bias=sb_k)
            rawm = bigs.tile([R, BL], f32)
            nc.vector.tensor_scalar_mul(rawm, raw, sb_msk)

            # denom = sum_r rawm  (fp32 K=64 matmul with ones)
            ones_r = singles.tile([R, 1], f32)
            nc.vector.memset(ones_r, 1.0)
            ps_den = ps_small.tile([1, BL], f32)
            for h in range(2):
                sl = slice(h * 512, (h + 1) * 512)
                nc.tensor.matmul(
                    ps_den[:, sl], ones_r, rawm[:, sl], start=True, stop=True
                )
            eps_1 = singles.tile([1, 1], f32)
            nc.vector.memset(eps_1, 1e-10)
            lnd = singles.tile([1, BL], f32)
            nc.scalar.activation(lnd, ps_den, AF.Ln, bias=eps_1)
            # 1/denom = exp(-ln(denom)); broadcast to all 64 rule rows via a
            # DRAM-bounce DMA (compute engines cannot partition-broadcast).
            recip = singles.tile([1, BL], f32)
            nc.scalar.activation(recip, lnd, AF.Exp, scale=-1.0)
            dram_cm = tc.tile_pool(name="dram", bufs=1, space="DRAM")
            drams = dram_cm.__enter__()
            recip_dram = drams.tile([1, BL], f32)
            nc.sync.dma_start(out=recip_dram, in_=recip)
            recip_rep = bigs.tile([R, BL], f32)
            nc.sync.dma_start(
                out=recip_rep, in_=recip_dram[0:1, :].to_broadcast((R, BL))
            )
            # frs^T (bf16) = rawm * (1/denom)  (normalised in fp32, cast bf16)
            frsm = bigs.tile([R, BL], bf16)
            nc.vector.tensor_mul(frsm, rawm, recip_rep)

            # ---- bf16 copies of the GEMM operands ----------------------
            wst_bf = bigs.tile([D, R * C], bf16)
            nc.scalar.copy(wst_bf, sb_wst)
            b2d_bf = singles.tile([R, C], bf16)
            nc.vector.tensor_copy(b2d_bf, sb_b2d)

            # ---- frs replicas: SBUF row -> DRAM -> broadcast to 128 parts
            frs_dram = drams.tile([R, BL], bf16)
            nc.sync.dma_start(out=frs_dram, in_=frsm)

            # ---- gated GEMM: out^T[c,b] accumulated over rules ----------
            ps_early_cm.__exit__(None, None, None)
            ps_acc_cm = tc.tile_pool(name="ps_acc", bufs=1, space="PSUM")
            ps_accp = ps_acc_cm.__enter__()
            ps_out = ps_accp.tile([C, BL], f32)
            with tc.tile_pool(name="reps", bufs=8) as reps:
                for r in range(R):
                    rep = reps.tile([D, BL], bf16)
                    nc.sync.dma_start(
                        out=rep,
                        in_=frs_dram[r : r + 1, :].to_broadcast((D, BL)),
                    )
                    g = gpool.tile([D, BL], bf16)
                    nc.vector.tensor_mul(g, xn_bf, rep)
                    for h in range(2):
                        sl = slice(h * 512, (h + 1) * 512)
                        nc.tensor.matmul(
                            ps_out[:, sl],
                            wst_bf[:, r * C : (r + 1) * C],
                            g[:, sl],
                            start=(r == 0),
                            stop=False,
                        )
            # bias term: out^T += biases2d^T @ frs^T  (closes the group)
            for h in range(2):
                sl = slice(h * 512, (h + 1) * 512)
                nc.tensor.matmul(
                    ps_out[:, sl], b2d_bf, frsm[:, sl], start=False, stop=True
                )

            # ---- evacuate + store --------------------------------------
            outf = bigs.tile([C, BL], f32)
            nc.vector.tensor_copy(outf, ps_out)
            nc.sync.dma_start(out=d_out, in_=outf)
            ps_acc_cm.__exit__(None, None, None)
            dram_cm.__exit__(None, None, None)

    nc.compile()
    return nc


def _get_nc():
    if "nc" not in _CACHE:
        _CACHE["nc"] = _build_bass()
    return _CACHE["nc"]


def _host_prep(x, centers, sigmas, weights, biases, bn_gamma, bn_beta, rule_masks):
    xT = np.ascontiguousarray(np.asarray(x, dtype=np.float32).T)  # [D, B]
    wstack = np.ascontiguousarray(
        np.transpose(np.asarray(weights, dtype=np.float32), (1, 0, 2)).reshape(D, R * C)
    )
    common = {
        "xt_full": xT,
        "centers_t": np.ascontiguousarray(np.asarray(centers, np.float32)),
        "sigmas_t": np.ascontiguousarray(np.asarray(sigmas, np.float32)),
        "wstack": wstack,
        "biases2d": np.ascontiguousarray(np.asarray(biases, np.float32)[0]),
        "gamma_c": np.ascontiguousarray(np.asarray(bn_gamma, np.float32).reshape(D, 1)),
        "beta_c": np.ascontiguousarray(np.asarray(bn_beta, np.float32).reshape(D, 1)),
        "masks_c": np.ascontiguousarray(np.asarray(rule_masks, np.float32).reshape(R, 1)),
    }
    in_maps = []
    for m in range(NCORES):
        im = dict(common)
        im["xt_loc"] = np.ascontiguousarray(xT[:, m * BL : (m + 1) * BL])
        in_maps.append(im)
    return in_maps


def run_on_hw(inputs, trace=False, **kw):
    from concourse.bass_utils import run_bass_kernel_spmd

    nc = _get_nc()
    in_maps = _host_prep(**inputs)
    res = run_bass_kernel_spmd(
        nc, in_maps, core_ids=list(range(NCORES)), trace=trace, **kw
    )
    out = np.empty((B, C), dtype=np.float32)
    for m in range(NCORES):
        out[m * BL : (m + 1) * BL, :] = res.results[m]["outT"].T
    return out, res


def kernel(x, centers, sigmas, weights, biases, bn_gamma, bn_beta, rule_masks):
    out, _ = run_on_hw(
        dict(
            x=x, centers=centers, sigmas=sigmas, weights=weights, biases=biases,
            bn_gamma=bn_gamma, bn_beta=bn_beta, rule_masks=rule_masks,
        )
    )
    return out
